# revision 1
# baseline (speedup 1.0000x reference)
"""Bass/Tile kernel for nn_CellTypeSpecEmbedding on TRN2 (8 cores, data-parallel).

Per core: 8 batch items -> 32 sequences of 128 tokens, 3 graphormer layers.
Feature-major activations [128p, 2chunk, T]; fp32r matmuls for N>=256,
bf16 attention internals; transposed-logits layout (no attention transposes);
LayerNorm stats via ones-matmul partition reductions + PE broadcast;
softmax denominators via ones-column appended to V.
"""
import sys
sys.path.insert(0, '/opt/trn_rl_repo')
import numpy as np
import ml_dtypes

import concourse.bass as bass
import concourse.mybir as mybir
import concourse.tile as tile
from concourse import bacc
from concourse.masks import make_identity

F32 = mybir.dt.float32
F32R = mybir.dt.float32r
BF16 = mybir.dt.bfloat16
AF = mybir.ActivationFunctionType
OP = mybir.AluOpType

B_CORE = 8       # batch items per core
G = 4
K = 128
D = 256
H = 8
DH = 32
L = 3
DFF = 1024
NSEQ = B_CORE * G          # 32
GROUPS = NSEQ // 4         # 8 groups of 4 seqs
T = 4 * K                  # 512 tokens per group
NT = NSEQ * K              # 4096 tokens per core

BIAS_NAMES = ["lin_b", "bq", "bk", "bv", "bo", "b1", "b2", "ln1_b", "ln2_b"]


def r32(ap):
    return ap.bitcast(F32R)


def build_nc(flags, ablate=()):
    ablate = frozenset(ablate)
    """flags: dict name->bool for optional bias tensors."""
    nc = bacc.Bacc("TRN2", target_bir_lowering=False, debug=False)

    def din(name, shape, dt=F32):
        return nc.dram_tensor(name, shape, dt, kind="ExternalInput").ap()

    featT_d = din("featT", [D, NT])
    centrT_d = din("centrT", [D, NT])
    hopT_d = din("hopT", [NSEQ, K, K])
    linW_d = din("lin_W", [D, D])
    Wq_d = din("Wq", [L, D, D])
    Wk_d = din("Wk", [L, D, D])
    Wv_d = din("Wv", [L, D, D])
    Wo_d = din("Wo", [L, D, D], BF16)
    W1_d = din("W1", [L, D, DFF])
    W2_d = din("W2", [L, DFF, D], BF16)
    fusW_d = din("fus_W", [D, D])
    fusv_d = din("fus_v", [128, 2])
    csb_d = din("csb", [128, L, H])
    g1_d = din("g1", [128, L, 2])
    g2_d = din("g2", [128, L, 2])
    bias_d = {}
    bias_shapes = {
        "lin_b": [128, 2], "bq": [128, L, 2], "bk": [128, L, 2],
        "bo": [128, L, 2], "b1": [128, L, 8], "b2": [128, L, 2],
        "ln1_b": [128, L, 2], "ln2_b": [128, L, 2], "bv": [L, D],
    }
    for nm in BIAS_NAMES:
        if flags[nm]:
            bias_d[nm] = din(nm, bias_shapes[nm])

    out_d = nc.dram_tensor("outT", [128, 2, B_CORE], F32, kind="ExternalOutput").ap()

    with tile.TileContext(nc) as tc, \
         tc.tile_pool(name="wconst", bufs=1) as wp, \
         tc.tile_pool(name="io", bufs=2) as iop, \
         tc.tile_pool(name="act", bufs=2) as actp, \
         tc.tile_pool(name="bf", bufs=2) as bfp, \
         tc.tile_pool(name="seq", bufs=3) as seqp, \
         tc.tile_pool(name="small", bufs=2) as smp, \
         tc.tile_pool(name="ps", bufs=8, space="PSUM") as pp:

        # ---- persistent constants ----
        def load_w(tag, dram, shape, pat, dt=F32, rnd=False):
            t = wp.tile(shape, dt, tag=tag)
            src_ap = dram.rearrange(pat, p=128)
            if rnd:
                nc.sync.dma_start(out=r32(t), in_=src_ap.bitcast(F32R))
            else:
                nc.sync.dma_start(out=t, in_=src_ap)
            return t

        def alloc_wl(tag, shape, dt=F32):
            return wp.tile(shape, dt, tag=tag, name=tag)

        lin_sb = load_w("lin_sb", linW_d, [128, 2, D], "(c p) o -> p c o", rnd=True)
        wq_sb = alloc_wl("wq_sb", [128, L, 2, D])
        wk_sb = alloc_wl("wk_sb", [128, L, 2, D])
        wv_sb = alloc_wl("wv_sb", [128, L, 2, D])
        wo_sb = alloc_wl("wo_sb", [128, L, 2, D], BF16)
        w1_sb = alloc_wl("w1_sb", [128, L, 2, DFF])
        w2_sb = alloc_wl("w2_sb", [128, L, 8, D], BF16)
        for l in range(L):
            for t_sb, dram, pat, rnd in (
                (wq_sb, Wq_d, "(c p) o -> p c o", True),
                (wk_sb, Wk_d, "(c p) o -> p c o", True),
                (wv_sb, Wv_d, "(c p) o -> p c o", True),
                (wo_sb, Wo_d, "(c p) o -> p c o", False),
                (w1_sb, W1_d, "(c p) o -> p c o", True),
                (w2_sb, W2_d, "(f p) o -> p f o", False),
            ):
                src_ap = dram[l].rearrange(pat, p=128)
                if rnd:
                    nc.sync.dma_start(out=r32(t_sb[:, l]), in_=src_ap.bitcast(F32R))
                else:
                    nc.sync.dma_start(out=t_sb[:, l], in_=src_ap)
        fusw_sb = load_w("fusw_sb", fusW_d, [128, 2, D], "(c p) o -> p c o", rnd=True)

        def load_small(tag, dram, shape):
            t = wp.tile(shape, F32, tag=tag)
            nc.sync.dma_start(out=t, in_=dram)
            return t

        fusv_sb = wp.tile([128, 2], F32, tag="fusv_sb")
        nc.sync.dma_start(out=r32(fusv_sb), in_=fusv_d.bitcast(F32R))
        c_sb = load_small("c_sb", csb_d, [128, L, H])
        g1_sb = load_small("g1_sb", g1_d, [128, L, 2])
        g2_sb = load_small("g2_sb", g2_d, [128, L, 2])

        bias_sb = {}
        for nm in BIAS_NAMES:
            if not flags[nm]:
                continue
            if nm == "bv":
                bvb = wp.tile([128, L, D], F32, tag="bias_bv")
                src = bass.AP(tensor=bias_d[nm].tensor, offset=bias_d[nm].offset,
                              ap=[[0, 128]] + list(bias_d[nm].ap))
                nc.sync.dma_start(out=bvb, in_=src)
                bias_sb[nm] = bvb
            else:
                bias_sb[nm] = load_small(f"bias_{nm}", bias_d[nm], bias_shapes[nm])

        ident_bf = wp.tile([128, 128], BF16, tag="ident_bf")
        make_identity(nc, ident_bf)
        ones_col = wp.tile([128, 1], F32, tag="ones_col")
        nc.vector.memset(ones_col, 1.0)
        nc.vector.tensor_scalar_mul(r32(ones_col), ones_col, 1.0)
        ones_row = wp.tile([1, 128], F32, tag="ones_row")
        nc.vector.memset(ones_row, 1.0)
        nc.vector.tensor_scalar_mul(r32(ones_row), ones_row, 1.0)
        eps_sb = wp.tile([1, 1], F32, tag="eps_sb")
        nc.vector.memset(eps_sb, 1e-5)
        xcls = wp.tile([128, 2, NSEQ], F32, tag="xcls")

        featT_r = featT_d.rearrange("(c p) t -> p c t", p=128)
        centrT_r = centrT_d.rearrange("(c p) t -> p c t", p=128)

        def layer_norm(u, y, g_col, b_col):
            """u,y: [128,2,T] f32; g_col: [128,2] slice; b_col: [128,2] slice or None."""
            if "ln" in ablate:
                for c in range(2):
                    nc.vector.tensor_copy(r32(y[:, c]), u[:, c])
                return
            ps1 = pp.tile([128, T], F32, tag="ps")
            for c in range(2):
                nc.tensor.matmul(ps1[0:1, :], r32(ones_col), r32(u[:, c]),
                                 start=(c == 0), stop=(c == 1))
            usq = actp.tile([128, 2, T], F32, tag="usq")
            for c in range(2):
                nc.scalar.activation(out=r32(usq[:, c]), in_=u[:, c], func=AF.Square)
            ps2 = pp.tile([128, T], F32, tag="ps")
            for c in range(2):
                nc.tensor.matmul(ps2[0:1, :], r32(ones_col), r32(usq[:, c]),
                                 start=(c == 0), stop=(c == 1))
            m = smp.tile([1, T], F32, tag="ln_m")
            nc.vector.tensor_scalar_mul(r32(m), ps1[0:1, :], 1.0 / D)
            m2 = smp.tile([1, T], F32, tag="ln_m2")
            nc.scalar.activation(out=m2, in_=m, func=AF.Square)
            q2 = smp.tile([1, T], F32, tag="ln_q2")
            nc.vector.scalar_tensor_tensor(
                out=q2, in0=ps2[0:1, :], scalar=1.0 / D, in1=m2,
                op0=OP.mult, op1=OP.subtract)
            nc.scalar.activation(out=q2, in_=q2, func=AF.Sqrt, bias=eps_sb)
            rstd = smp.tile([1, T], F32, tag="ln_rstd")
            with nc.allow_low_precision(reason="tf32 rstd is within error budget"):
                nc.vector.reciprocal(r32(rstd), q2)
            psm = pp.tile([128, T], F32, tag="ps")
            nc.tensor.matmul(psm, r32(ones_row), r32(m))
            psr = pp.tile([128, T], F32, tag="ps")
            nc.tensor.matmul(psr, r32(ones_row), r32(rstd))
            for c in range(2):
                nc.vector.tensor_tensor(out=r32(y[:, c]), in0=u[:, c], in1=psm, op=OP.subtract)
                last_out = r32(y[:, c])
                nc.vector.scalar_tensor_tensor(
                    out=last_out, in0=y[:, c], scalar=g_col[:, c:c + 1], in1=psr,
                    op0=OP.mult, op1=OP.mult)  # noqa
                if b_col is not None:
                    nc.vector.tensor_scalar_add(r32(y[:, c]), y[:, c], b_col[:, c:c + 1])

        # =================== main loop over group pairs ===================
        # Two groups are processed layer-phase-interleaved so one group's
        # PE-sparse attention overlaps the other's PE-dense FFN/LN.

        def load_group(g):
            tok0 = g * T
            ft = iop.tile([128, 2, T], F32, tag="ft", name=f"ft{g}")
            nc.scalar.dma_start(out=ft, in_=featT_r[:, :, tok0:tok0 + T])
            ct = iop.tile([128, 2, T], F32, tag="ct", name=f"ct{g}")
            nc.scalar.dma_start(out=ct, in_=centrT_r[:, :, tok0:tok0 + T])
            hop_g = iop.tile([128, 4, K], F32, tag="hop", name=f"hop{g}")
            nc.scalar.dma_start(
                out=hop_g,
                in_=hopT_d[g * 4:(g + 1) * 4].rearrange("s k q -> k s q"))
            xin = actp.tile([128, 2, T], F32, tag="xin", name=f"xin{g}")
            for c in range(2):
                nc.vector.scalar_tensor_tensor(
                    out=r32(xin[:, c]), in0=ft[:, c], scalar=16.0, in1=ct[:, c],
                    op0=OP.mult, op1=OP.add)
            xT = actp.tile([128, 2, T], F32, tag="xT", name=f"x0_{g}")
            for c in range(2):
                ps = pp.tile([128, T], F32, tag="ps", name=f"pslin{g}{c}")
                for ci in range(2):
                    nc.tensor.matmul(ps, r32(lin_sb[:, ci, c * 128:(c + 1) * 128]),
                                     r32(xin[:, ci]), start=(ci == 0), stop=(ci == 1))
                bias = bias_sb["lin_b"][:, c:c + 1] if flags["lin_b"] else 0.0
                nc.scalar.activation(out=r32(xT[:, c]), in_=ps, func=AF.Relu, bias=bias)
            return {"xT": xT, "hop": hop_g, "g": g}

        def attention(st, l):
            xT, hop_g, g = st["xT"], st["hop"], st["g"]
            q_sb = bfp.tile([128, 2, T], BF16, tag="q_sb", name=f"q{g}{l}")
            k_sb = bfp.tile([128, 2, T], BF16, tag="k_sb", name=f"k{g}{l}")
            if "qkv" in ablate:
                nc.vector.memset(q_sb, 0.01)
                nc.vector.memset(k_sb, 0.01)
            for c in (() if "qkv" in ablate else range(2)):
                psq = pp.tile([128, T], F32, tag="ps", name=f"psq{g}{l}{c}")
                for ci in range(2):
                    nc.tensor.matmul(psq, r32(wq_sb[:, l, ci, c * 128:(c + 1) * 128]),
                                     r32(xT[:, ci]), start=(ci == 0), stop=(ci == 1))
                psk = pp.tile([128, T], F32, tag="ps", name=f"psk{g}{l}{c}")
                for ci in range(2):
                    nc.tensor.matmul(psk, r32(wk_sb[:, l, ci, c * 128:(c + 1) * 128]),
                                     r32(xT[:, ci]), start=(ci == 0), stop=(ci == 1))
                if flags["bq"]:
                    nc.vector.tensor_scalar_add(q_sb[:, c], psq, bias_sb["bq"][:, l, c:c + 1])
                else:
                    nc.vector.tensor_copy(q_sb[:, c], psq)
                if flags["bk"]:
                    nc.vector.tensor_scalar_add(k_sb[:, c], psk, bias_sb["bk"][:, l, c:c + 1])
                else:
                    nc.vector.tensor_copy(k_sb[:, c], psk)

            ctxT_g = bfp.tile([128, 2, T], BF16, tag="ctxT_g", name=f"ctxT{g}{l}")
            if "attn" in ablate:
                nc.vector.memset(ctxT_g, 0.01)
            for s in (() if "attn" in ablate else range(4)):
                scol = slice(s * K, (s + 1) * K)
                psv = pp.tile([128, D], F32, tag="ps", name=f"psv{g}{l}{s}")
                for ci in range(2):
                    nc.tensor.matmul(psv, r32(xT[:, ci, scol]),
                                     r32(wv_sb[:, l, ci, :]),
                                     start=(ci == 0), stop=(ci == 1))
                psv_h = psv.rearrange("p (h e) -> p h e", h=H)
                v_sb = seqp.tile([128, H, 33], BF16, tag="v_sb", name=f"v{g}{l}{s}")
                if flags["bv"]:
                    nc.vector.tensor_tensor(
                        out=v_sb[:, :, 0:32], in0=psv_h,
                        in1=bias_sb["bv"][:, l].rearrange("p (h e) -> p h e", h=H),
                        op=OP.add)
                else:
                    nc.vector.tensor_copy(v_sb[:, :, 0:32], psv_h)
                nc.vector.memset(v_sb[:, :, 32:33], 1.0)

                lg_sb = seqp.tile([128, H, K], F32, tag="lg_sb", name=f"lg{g}{l}{s}")
                for h in range(H):
                    hc, hp = h // 4, (h % 4) * 32
                    pslg = pp.tile([128, K], F32, tag="ps", name=f"pslg{g}{l}{s}{h}")
                    nc.tensor.matmul(pslg,
                                     k_sb[hp:hp + 32, hc, scol],
                                     q_sb[hp:hp + 32, hc, scol],
                                     tile_position=(hp, 0))
                    nc.vector.scalar_tensor_tensor(
                        out=lg_sb[:, h], in0=hop_g[:, s],
                        scalar=c_sb[:, l, h:h + 1],
                        in1=pslg, op0=OP.mult, op1=OP.add)
                E = seqp.tile([128, H, K], BF16, tag="E", name=f"E{g}{l}{s}")
                nc.scalar.activation(out=E[:, 0:4], in_=lg_sb[:, 0:4], func=AF.Exp)
                nc.scalar.activation(out=E[:, 4:8], in_=lg_sb[:, 4:8], func=AF.Exp)

                psc = pp.tile([128, H, 33], F32, tag="ps", name=f"psc{g}{l}{s}")
                for h in range(H):
                    nc.tensor.matmul(psc[:, h], E[:, h], v_sb[:, h])
                rs = seqp.tile([128, H], F32, tag="rs", name=f"rs{g}{l}{s}")
                nc.vector.reciprocal(rs, psc[:, :, 32])
                ctx_sb = seqp.tile([128, H, 32], BF16, tag="ctx_sb", name=f"cx{g}{l}{s}")
                nc.vector.scalar_tensor_tensor(
                    out=ctx_sb, in0=psc[:, :, 0:32], scalar=1.0,
                    in1=rs[:, :, None].to_broadcast((128, H, 32)),
                    op0=OP.mult, op1=OP.mult)
                ctx_flat = ctx_sb.rearrange("p h e -> p (h e)")
                for c in range(2):
                    pst = pp.tile([128, 128], BF16, tag="ps", name=f"pst{g}{l}{s}{c}")
                    nc.tensor.transpose(pst, ctx_flat[:, c * 128:(c + 1) * 128],
                                        ident_bf)
                    nc.vector.tensor_copy(ctxT_g[:, c, scol], pst)
            st["ctxT"] = ctxT_g

        def o_ln1(st, l):
            xT, ctxT_g, g = st["xT"], st["ctxT"], st["g"]
            u1 = actp.tile([128, 2, T], F32, tag="u1", name=f"u1_{g}{l}")
            for c in range(2):
                pso = pp.tile([128, T], F32, tag="ps", name=f"pso{g}{l}{c}")
                for ci in range(2):
                    nc.tensor.matmul(pso, wo_sb[:, l, ci, c * 128:(c + 1) * 128],
                                     ctxT_g[:, ci], start=(ci == 0), stop=(ci == 1))
                if flags["bo"]:
                    nc.vector.scalar_tensor_tensor(
                        out=r32(u1[:, c]), in0=pso, scalar=bias_sb["bo"][:, l, c:c + 1],
                        in1=xT[:, c], op0=OP.add, op1=OP.add)
                else:
                    nc.vector.tensor_tensor(out=r32(u1[:, c]), in0=pso, in1=xT[:, c], op=OP.add)
            y1 = actp.tile([128, 2, T], F32, tag="y1", name=f"y1_{g}{l}")
            layer_norm(u1, y1, g1_sb[:, l],
                       bias_sb["ln1_b"][:, l] if flags["ln1_b"] else None)
            st["y1"] = y1

        def ffn_ln2(st, l):
            y1, g = st["y1"], st["g"]
            f1 = bfp.tile([128, 8, T], BF16, tag="f1", name=f"f1_{g}{l}")
            for f in (() if "ffn" in ablate else range(8)):
                psf = pp.tile([128, T], F32, tag="ps", name=f"psf{g}{l}{f}")
                for ci in range(2):
                    nc.tensor.matmul(psf, r32(w1_sb[:, l, ci, f * 128:(f + 1) * 128]),
                                     r32(y1[:, ci]), start=(ci == 0), stop=(ci == 1))
                bias = bias_sb["b1"][:, l, f:f + 1] if flags["b1"] else 0.0
                nc.scalar.activation(out=f1[:, f], in_=psf, func=AF.Relu, bias=bias)
            u2 = actp.tile([128, 2, T], F32, tag="u2", name=f"u2_{g}{l}")
            if "ffn" in ablate:
                for c in range(2):
                    nc.vector.tensor_copy(r32(u2[:, c]), y1[:, c])
            for c in (() if "ffn" in ablate else range(2)):
                ps2 = pp.tile([128, T], F32, tag="ps", name=f"ps2{g}{l}{c}")
                for f in range(8):
                    nc.tensor.matmul(ps2, w2_sb[:, l, f, c * 128:(c + 1) * 128],
                                     f1[:, f], start=(f == 0), stop=(f == 7))
                if flags["b2"]:
                    nc.vector.scalar_tensor_tensor(
                        out=r32(u2[:, c]), in0=ps2, scalar=bias_sb["b2"][:, l, c:c + 1],
                        in1=y1[:, c], op0=OP.add, op1=OP.add)
                else:
                    nc.vector.tensor_tensor(out=r32(u2[:, c]), in0=ps2, in1=y1[:, c], op=OP.add)
            xT = actp.tile([128, 2, T], F32, tag="xT", name=f"x{g}{l}")
            layer_norm(u2, xT, g2_sb[:, l],
                       bias_sb["ln2_b"][:, l] if flags["ln2_b"] else None)
            st["xT"] = xT

        for g in range(GROUPS):
            st = load_group(g)
            for l in range(L):
                attention(st, l)
                o_ln1(st, l)
                ffn_ln2(st, l)
            nc.vector.tensor_copy(
                r32(xcls[:, :, g * 4:(g + 1) * 4]),
                st["xT"].rearrange("p c (s t) -> p c s t", t=K)[:, :, :, 0])

        # =================== fusion head ===================
        pst = [pp.tile([128, NSEQ], F32, tag="ps", name=f"pst{c}") for c in range(2)]
        for c in range(2):
            for ci in range(2):
                nc.tensor.matmul(pst[c], r32(fusw_sb[:, ci, c * 128:(c + 1) * 128]),
                                 r32(xcls[:, ci]), start=(ci == 0), stop=(ci == 1))
        th = smp.tile([128, 2, NSEQ], F32, tag="th")
        for c in range(2):
            nc.scalar.activation(out=r32(th[:, c]), in_=pst[c], func=AF.Tanh)
        pssc = pp.tile([128, NSEQ], F32, tag="ps")
        for ci in range(2):
            nc.tensor.matmul(pssc[0:1, :], r32(fusv_sb[:, ci:ci + 1]), r32(th[:, ci]),
                             start=(ci == 0), stop=(ci == 1))
        es = smp.tile([1, NSEQ], F32, tag="es")
        nc.scalar.activation(out=es, in_=pssc[0:1, :], func=AF.Exp)
        esv = es.rearrange("o (b g) -> o b g", g=4)
        s01 = smp.tile([1, B_CORE], F32, tag="s01")
        nc.vector.tensor_tensor(out=s01, in0=esv[:, :, 0], in1=esv[:, :, 1], op=OP.add)
        s23 = smp.tile([1, B_CORE], F32, tag="s23")
        nc.vector.tensor_tensor(out=s23, in0=esv[:, :, 2], in1=esv[:, :, 3], op=OP.add)
        stot = smp.tile([1, B_CORE], F32, tag="stot")
        nc.vector.tensor_tensor(out=stot, in0=s01, in1=s23, op=OP.add)
        rtot = smp.tile([1, B_CORE], F32, tag="rtot")
        nc.vector.reciprocal(rtot, stot)
        w_sm = smp.tile([1, NSEQ], F32, tag="w_sm")
        nc.vector.tensor_tensor(
            out=r32(w_sm.rearrange("o (b g) -> o b g", g=4)), in0=esv,
            in1=rtot[:, :, None].to_broadcast((1, B_CORE, 4)), op=OP.mult)
        pswb = pp.tile([128, NSEQ], F32, tag="ps")
        nc.tensor.matmul(pswb, r32(ones_row), r32(w_sm))
        wx = smp.tile([128, 2, B_CORE, 4], F32, tag="wx")
        nc.vector.tensor_tensor(
            out=wx, in0=xcls.rearrange("p c (b g) -> p c b g", g=4),
            in1=pswb.rearrange("p (b g) -> p b g", g=4)[:, None].to_broadcast(
                (128, 2, B_CORE, 4)),
            op=OP.mult)
        o01 = smp.tile([128, 2, B_CORE], F32, tag="o01")
        nc.vector.tensor_tensor(out=o01, in0=wx[:, :, :, 0], in1=wx[:, :, :, 1], op=OP.add)
        o23 = smp.tile([128, 2, B_CORE], F32, tag="o23")
        nc.vector.tensor_tensor(out=o23, in0=wx[:, :, :, 2], in1=wx[:, :, :, 3], op=OP.add)
        outT = smp.tile([128, 2, B_CORE], F32, tag="outT")
        nc.vector.tensor_tensor(out=outT, in0=o01, in1=o23, op=OP.add)
        nc.sync.dma_start(out=out_d, in_=outT)

    nc.compile()
    return nc


# ======================= host side =======================

def host_prep(inputs):
    """Full inputs -> (flags, per-core in_maps list)."""
    f32 = np.float32
    node_id = np.asarray(inputs["node_id"])
    nbr_tab = np.asarray(inputs["neighbor_table"])
    deg_tab = np.asarray(inputs["degree_table"])
    feat_tab = np.asarray(inputs["node_feat_table"], dtype=f32)
    centr_tab = np.asarray(inputs["centr_table"], dtype=f32)
    sp_tab = np.asarray(inputs["spatial_table"], dtype=f32)

    spW1 = np.asarray(inputs["spW1"], dtype=f32)
    spb1 = np.asarray(inputs["spb1"], dtype=f32)
    spW2 = np.asarray(inputs["spW2"], dtype=f32)
    assert np.all(spb1 == 0.0), "kernel assumes spb1 == 0 (as in setup_inputs)"
    c_coef = np.einsum("ld,ldh->lh", np.maximum(spW1[:, 0, :], 0.0), spW2)  # [L,H]

    Wq = np.asarray(inputs["Wq"], dtype=f32) / np.sqrt(np.float32(DH))
    bq = np.asarray(inputs["bq"], dtype=f32) / np.sqrt(np.float32(DH))
    Wk = np.asarray(inputs["Wk"], dtype=f32)
    Wv = np.asarray(inputs["Wv"], dtype=f32)
    Wo = np.asarray(inputs["Wo"], dtype=f32).astype(ml_dtypes.bfloat16)
    W1 = np.asarray(inputs["ffn_W1"], dtype=f32)
    W2 = np.asarray(inputs["ffn_W2"], dtype=f32).astype(ml_dtypes.bfloat16)

    def pmaj(v, cols):   # [X] -> [128, X/128] partition-major
        return np.ascontiguousarray(v.reshape(cols, 128).T)

    def pmaj_l(v, cols):  # [L, X] -> [128, L, X/128]
        return np.ascontiguousarray(v.reshape(L, cols, 128).transpose(2, 0, 1))

    flags = {
        "lin_b": bool(np.any(np.asarray(inputs["lin_b"]) != 0)),
        "bq": bool(np.any(bq != 0)),
        "bk": bool(np.any(np.asarray(inputs["bk"]) != 0)),
        "bv": bool(np.any(np.asarray(inputs["bv"]) != 0)),
        "bo": bool(np.any(np.asarray(inputs["bo"]) != 0)),
        "b1": bool(np.any(np.asarray(inputs["ffn_b1"]) != 0)),
        "b2": bool(np.any(np.asarray(inputs["ffn_b2"]) != 0)),
        "ln1_b": bool(np.any(np.asarray(inputs["ln1_b"]) != 0)),
        "ln2_b": bool(np.any(np.asarray(inputs["ln2_b"]) != 0)),
    }

    shared = {
        "lin_W": np.ascontiguousarray(np.asarray(inputs["lin_W"], dtype=f32)),
        "Wq": np.ascontiguousarray(Wq), "Wk": np.ascontiguousarray(Wk),
        "Wv": np.ascontiguousarray(Wv), "Wo": np.ascontiguousarray(Wo),
        "W1": np.ascontiguousarray(W1), "W2": np.ascontiguousarray(W2),
        "fus_W": np.ascontiguousarray(np.asarray(inputs["fus_W"], dtype=f32)),
        "fus_v": pmaj(np.asarray(inputs["fus_v"], dtype=f32), 2),
        "csb": np.ascontiguousarray(
            np.broadcast_to(c_coef[None, :, :], (128, L, H)).astype(f32)),
        "g1": pmaj_l(np.asarray(inputs["ln1_g"], dtype=f32), 2),
        "g2": pmaj_l(np.asarray(inputs["ln2_g"], dtype=f32), 2),
    }
    if flags["lin_b"]:
        shared["lin_b"] = pmaj(np.asarray(inputs["lin_b"], dtype=f32), 2)
    if flags["bq"]:
        shared["bq"] = pmaj_l(bq, 2)
    if flags["bk"]:
        shared["bk"] = pmaj_l(np.asarray(inputs["bk"], dtype=f32), 2)
    if flags["bv"]:
        shared["bv"] = np.ascontiguousarray(np.asarray(inputs["bv"], dtype=f32))
    if flags["bo"]:
        shared["bo"] = pmaj_l(np.asarray(inputs["bo"], dtype=f32), 2)
    if flags["b1"]:
        shared["b1"] = pmaj_l(np.asarray(inputs["ffn_b1"], dtype=f32), 8)
    if flags["b2"]:
        shared["b2"] = pmaj_l(np.asarray(inputs["ffn_b2"], dtype=f32), 2)
    if flags["ln1_b"]:
        shared["ln1_b"] = pmaj_l(np.asarray(inputs["ln1_b"], dtype=f32), 2)
    if flags["ln2_b"]:
        shared["ln2_b"] = pmaj_l(np.asarray(inputs["ln2_b"], dtype=f32), 2)

    in_maps = []
    for core in range(8):
        ids = node_id[core * B_CORE:(core + 1) * B_CORE]
        idx = nbr_tab[ids].reshape(-1)                       # [4096]
        featT = np.ascontiguousarray(np.nan_to_num(feat_tab[idx]).T)   # [256,4096]
        deg = deg_tab[idx, 0]
        centrT = np.ascontiguousarray(centr_tab[deg].T)
        hopT = np.ascontiguousarray(
            sp_tab[ids][:, :, 0].transpose(0, 1, 3, 2).reshape(NSEQ, K, K))
        m = dict(shared)
        m["featT"] = featT
        m["centrT"] = centrT
        m["hopT"] = hopT
        in_maps.append(m)
    return flags, in_maps


def assemble(results):
    """per-core outT [128, 2, 8] -> full [64, 256] f32."""
    outs = []
    for core in range(8):
        oT = results[core]["outT"]                # [128, 2, 8]
        outs.append(oT.transpose(2, 1, 0).reshape(B_CORE, D))
    return np.ascontiguousarray(np.concatenate(outs, 0).astype(np.float32))


# ======================= entry point =======================

import os as _os
_os.environ.setdefault("NEURON_RT_RESET_CORES", "1")

_BUILD_CACHE = {}


def kernel(**inputs):
    """Full (unsharded) inputs -> full [64, 256] float32 output.

    Shards the batch of 64 node_ids across 8 NeuronCores (8 per core),
    slices/gathers the lookup tables per core on the host, runs the Bass
    kernel on cores 0-7 via run_bass_kernel_spmd, and reassembles.
    """
    from concourse import bass_utils
    flags, in_maps = host_prep(inputs)
    key = tuple(sorted((k, v) for k, v in flags.items()))
    if key not in _BUILD_CACHE:
        _BUILD_CACHE[key] = build_nc(flags)
    nc = _BUILD_CACHE[key]
    res = bass_utils.run_bass_kernel_spmd(nc, in_maps, core_ids=list(range(8)))
    return assemble(res.results)



# revision 5
# speedup vs baseline: 1.4445x; 1.4445x over previous
"""Bass/Tile kernel v2 for nn_CellTypeSpecEmbedding on TRN2 (8 cores, data-parallel).

Structural changes vs v1:
- LN1 rstd eliminated: y1's per-token scale passes through the (bias-free)
  FFN and LN2 is invariant to per-token positive scaling -> only mean needed.
- hop spatial bias folded into the logits matmul via diag(c_lh) @ hop PSUM
  accumulation (removes all per-head DVE bias adds).
- LayerNorm mean broadcasts computed directly as [128,T] PSUM via J-matrix
  matmuls (J[k,p] = g[p]/D), removing [1,T] scalar-lane chains.
- rstd via exp(-0.5*ln(var+eps)) so all ACT funcs share one act table.
- Engine rebalance: PSUM evictions split ACT/DVE, SBUF-only work on Pool,
  per-group DMAs issued from the SP (sync) queue.
"""
import sys
sys.path.insert(0, '/opt/trn_rl_repo')
import numpy as np
import ml_dtypes

import concourse.bass as bass
import concourse.mybir as mybir
import concourse.tile as tile
from concourse import bacc
from concourse.masks import make_identity

F32 = mybir.dt.float32
F32R = mybir.dt.float32r
BF16 = mybir.dt.bfloat16
AF = mybir.ActivationFunctionType
OP = mybir.AluOpType

# ---- activation-table steering ----
# The act-table insertion pass maps each activation to the FIRST table
# containing its func, which thrashes between the exp table and the ln
# table (1.28us reload each). All funcs this kernel uses live together in
# natural_log_exp_and_others, so present the pass with a view where they
# appear ONLY there (indices preserved; every emitted act_func_set_id
# still names a real table containing the instruction's func).
_STEER = {AF.Exp, AF.Ln, AF.Relu, AF.Copy, AF.Identity, AF.Square}
_steer_cache = {}
import concourse.hw_specs as _hw_mod
_orig_get_act_tables = _hw_mod.get_activation_tables


def _steered_tables(arch):
    if arch not in _steer_cache:
        raw = _orig_get_act_tables(arch)
        out = {}
        for name, funcs in raw.items():
            if name == "natural_log_exp_and_others":
                out[name] = set(funcs)
            elif name == "exp_and_others":
                out[name] = set(funcs) - _STEER
            else:
                out[name] = set(funcs) - _STEER - {AF.Tanh}
        _steer_cache[arch] = out
    return _steer_cache[arch]


class _act_steering:
    """Scoped: steer only the bass-level act-table insertion during
    nc.compile(); the neuronxcc hook later sees the original tables and
    adopts the pre-placed loads."""

    def __enter__(self):
        import concourse.bacc as _bacc
        self._saved = _bacc.get_activation_tables
        _bacc.get_activation_tables = _steered_tables

    def __exit__(self, *exc):
        import concourse.bacc as _bacc
        _bacc.get_activation_tables = self._saved

B_CORE = 8       # batch items per core
G = 4
K = 128
D = 256
H = 8
DH = 32
L = 3
DFF = 1024
NSEQ = B_CORE * G          # 32
GROUPS = NSEQ // 4         # 8 groups of 4 seqs
T = 4 * K                  # 512 tokens per group
NT = NSEQ * K              # 4096 tokens per core
EPS = 1e-5

BIAS_NAMES = ["lin_b", "bq", "bk", "bv", "bo", "b1", "b2", "ln1_b", "ln2_b"]


def r32(ap):
    return ap.bitcast(F32R)


def build_nc(flags):
    """flags: dict name->bool for optional bias tensors."""
    nc = bacc.Bacc("TRN2", target_bir_lowering=False, debug=False)

    # z-trick (defer LN1's rstd) requires bias-free FFN + no ln1 bias
    ln1_fast = not (flags["b1"] or flags["b2"] or flags["ln1_b"])

    def din(name, shape, dt=F32):
        return nc.dram_tensor(name, shape, dt, kind="ExternalInput").ap()

    featT_d = din("featT", [D, NT])
    centrT_d = din("centrT", [D, NT])
    hopT_d = din("hopT", [NSEQ, K, K], BF16)
    linW_d = din("lin_W", [D, D])
    linW16_d = din("lin_W16", [D, D])
    Wq_d = din("Wq", [L, D, D])
    Wk_d = din("Wk", [L, D, D])
    Wv_d = din("Wv", [L, D, D])
    Wo_d = din("Wo", [L, D, D], BF16)
    W1_d = din("W1", [L, D, DFF])
    W2_d = din("W2", [L, DFF, D], BF16)
    fusW_d = din("fus_W", [D, D])
    fusv_d = din("fus_v", [128, 2])
    csb_d = din("csb", [128, L, H])
    jg1_d = din("Jg1", [128, L, 2, 128])
    g2r_d = din("g2row", [1, L, 2, 128])
    g1r_d = din("g1row", [1, L, 2, 128])
    g1c_d = din("g1col", [128, L, 2])
    bias_d = {}
    bias_shapes = {
        "lin_b": [128, 2], "bq": [128, L, 2], "bk": [128, L, 2],
        "bo": [128, L, 2], "b1": [128, L, 8], "b2": [128, L, 2],
        "ln1_b": [128, L, 2], "ln2_b": [128, L, 2], "bv": [L, D],
    }
    for nm in BIAS_NAMES:
        if flags[nm]:
            bias_d[nm] = din(nm, bias_shapes[nm])

    out_d = nc.dram_tensor("outT", [128, 2, B_CORE], F32, kind="ExternalOutput").ap()

    with tile.TileContext(nc) as tc, \
         tc.tile_pool(name="wconst", bufs=1) as wp, \
         tc.tile_pool(name="io", bufs=2) as iop, \
         tc.tile_pool(name="act", bufs=3) as actp, \
         tc.tile_pool(name="bf", bufs=2) as bfp, \
         tc.tile_pool(name="seq", bufs=4) as seqp, \
         tc.tile_pool(name="small", bufs=2) as smp, \
         tc.tile_pool(name="ps", bufs=8, space="PSUM") as pp:

        # ---- persistent constants ----
        def load_w(tag, dram, shape, pat, dt=F32, rnd=False):
            t = wp.tile(shape, dt, tag=tag)
            src_ap = dram.rearrange(pat, p=128)
            if rnd:
                nc.sync.dma_start(out=r32(t), in_=src_ap.bitcast(F32R))
            else:
                nc.sync.dma_start(out=t, in_=src_ap)
            return t

        def alloc_wl(tag, shape, dt=F32):
            return wp.tile(shape, dt, tag=tag, name=tag)

        lin_sb = load_w("lin_sb", linW_d, [128, 2, D], "(c p) o -> p c o", rnd=True)
        lin16_sb = load_w("lin16_sb", linW16_d, [128, 2, D], "(c p) o -> p c o", rnd=True)
        wq_sb = alloc_wl("wq_sb", [128, L, 2, D])
        wk_sb = alloc_wl("wk_sb", [128, L, 2, D])
        wv_sb = alloc_wl("wv_sb", [128, L, 2, D])
        wo_sb = alloc_wl("wo_sb", [128, L, 2, D], BF16)
        w1_sb = alloc_wl("w1_sb", [128, L, 2, DFF])
        w2_sb = alloc_wl("w2_sb", [128, L, 8, D], BF16)
        for l in range(L):
            for t_sb, dram, pat, rnd in (
                (wq_sb, Wq_d, "(c p) o -> p c o", True),
                (wk_sb, Wk_d, "(c p) o -> p c o", True),
                (wv_sb, Wv_d, "(c p) o -> p c o", True),
                (wo_sb, Wo_d, "(c p) o -> p c o", False),
                (w1_sb, W1_d, "(c p) o -> p c o", True),
                (w2_sb, W2_d, "(f p) o -> p f o", False),
            ):
                src_ap = dram[l].rearrange(pat, p=128)
                if rnd:
                    nc.sync.dma_start(out=r32(t_sb[:, l]), in_=src_ap.bitcast(F32R))
                else:
                    nc.sync.dma_start(out=t_sb[:, l], in_=src_ap)
        fusw_sb = load_w("fusw_sb", fusW_d, [128, 2, D], "(c p) o -> p c o", rnd=True)

        def load_small(tag, dram, shape):
            t = wp.tile(shape, F32, tag=tag)
            nc.sync.dma_start(out=t, in_=dram)
            return t

        fusv_sb = wp.tile([128, 2], F32, tag="fusv_sb")
        nc.sync.dma_start(out=r32(fusv_sb), in_=fusv_d.bitcast(F32R))
        c_sb = load_small("c_sb", csb_d, [128, L, H])
        jg1_sb = wp.tile([128, L, 2, 128], F32, tag="jg1_sb")
        nc.sync.dma_start(out=r32(jg1_sb), in_=jg1_d.bitcast(F32R))
        g2r_sb = wp.tile([1, L, 2, 128], F32, tag="g2r_sb")
        nc.sync.dma_start(out=r32(g2r_sb), in_=g2r_d.bitcast(F32R))
        g1r_sb = wp.tile([1, L, 2, 128], F32, tag="g1r_sb")
        nc.sync.dma_start(out=r32(g1r_sb), in_=g1r_d.bitcast(F32R))
        g1c_sb = load_small("g1c_sb", g1c_d, [128, L, 2])

        bias_sb = {}
        for nm in BIAS_NAMES:
            if not flags[nm]:
                continue
            if nm == "bv":
                bvb = wp.tile([128, L, D], F32, tag="bias_bv")
                src = bass.AP(tensor=bias_d[nm].tensor, offset=bias_d[nm].offset,
                              ap=[[0, 128]] + list(bias_d[nm].ap))
                nc.sync.dma_start(out=bvb, in_=src)
                bias_sb[nm] = bvb
            else:
                bias_sb[nm] = load_small(f"bias_{nm}", bias_d[nm], bias_shapes[nm])

        ident_bf = wp.tile([128, 128], BF16, tag="ident_bf")
        make_identity(nc, ident_bf)
        # diag(c_lh) in bf16, used to accumulate c_lh * hop into logits PSUM
        diag_sb = wp.tile([128, L, H, 128], BF16, tag="diag_sb")
        for l in range(L):
            for h in range(H):
                nc.vector.tensor_scalar_mul(diag_sb[:, l, h, :], ident_bf,
                                            c_sb[:, l, h:h + 1])
        onesd_col = wp.tile([128, 1], F32, tag="onesd_col")
        nc.vector.memset(onesd_col, 1.0 / D)
        nc.vector.tensor_scalar_mul(r32(onesd_col), onesd_col, 1.0)
        jpl = wp.tile([128, 128], F32, tag="jpl")
        nc.vector.memset(jpl, 1.0 / D)
        nc.vector.tensor_scalar_mul(r32(jpl), jpl, 1.0)
        ones_row = wp.tile([1, 128], F32, tag="ones_row")
        nc.vector.memset(ones_row, 1.0)
        nc.vector.tensor_scalar_mul(r32(ones_row), ones_row, 1.0)
        eps_sb = wp.tile([1, 1], F32, tag="eps_sb")
        nc.vector.memset(eps_sb, EPS)
        xcls = wp.tile([128, 2, NSEQ], F32, tag="xcls")

        featT_r = featT_d.rearrange("(c p) t -> p c t", p=128)
        centrT_r = centrT_d.rearrange("(c p) t -> p c t", p=128)

        # =================== per-group pipeline ===================

        def load_group(g):
            tok0 = g * T
            ft = iop.tile([128, 2, T], F32, tag="ft", name=f"ft{g}")
            nc.gpsimd.dma_start(out=r32(ft), in_=featT_r[:, :, tok0:tok0 + T].bitcast(F32R))
            ct = iop.tile([128, 2, T], F32, tag="ct", name=f"ct{g}")
            nc.gpsimd.dma_start(out=r32(ct), in_=centrT_r[:, :, tok0:tok0 + T].bitcast(F32R))
            hop_bf = iop.tile([128, 4, K], BF16, tag="hop", name=f"hop{g}")
            nc.gpsimd.dma_start(
                out=hop_bf,
                in_=hopT_d[g * 4:(g + 1) * 4].rearrange("s k q -> k s q"))
            xT = actp.tile([128, 2, T], F32, tag="xT", name=f"x0_{g}")
            for c in range(2):
                ps = pp.tile([128, T], F32, tag="ps", name=f"pslin{g}{c}")
                for ci in range(2):
                    nc.tensor.matmul(ps, r32(lin16_sb[:, ci, c * 128:(c + 1) * 128]),
                                     r32(ft[:, ci]), start=(ci == 0), stop=False)
                for ci in range(2):
                    nc.tensor.matmul(ps, r32(lin_sb[:, ci, c * 128:(c + 1) * 128]),
                                     r32(ct[:, ci]), start=False, stop=(ci == 1))
                bias = bias_sb["lin_b"][:, c:c + 1] if flags["lin_b"] else 0.0
                nc.scalar.activation(out=r32(xT[:, c]), in_=ps, func=AF.Relu, bias=bias)
            return {"xT": xT, "hop": hop_bf, "g": g}

        def attention(st, l):
            xT, hop_bf, g = st["xT"], st["hop"], st["g"]
            q_sb = bfp.tile([128, 2, T], BF16, tag="q_sb", name=f"q{g}{l}")
            k_sb = bfp.tile([128, 2, T], BF16, tag="k_sb", name=f"k{g}{l}")
            for c in range(2):
                psq = pp.tile([128, T], F32, tag="ps", name=f"psq{g}{l}{c}")
                for ci in range(2):
                    nc.tensor.matmul(psq, r32(wq_sb[:, l, ci, c * 128:(c + 1) * 128]),
                                     r32(xT[:, ci]), start=(ci == 0), stop=(ci == 1))
                psk = pp.tile([128, T], F32, tag="ps", name=f"psk{g}{l}{c}")
                for ci in range(2):
                    nc.tensor.matmul(psk, r32(wk_sb[:, l, ci, c * 128:(c + 1) * 128]),
                                     r32(xT[:, ci]), start=(ci == 0), stop=(ci == 1))
                # q eviction on ACT, k on DVE (balance)
                if flags["bq"]:
                    nc.scalar.activation(out=q_sb[:, c], in_=psq, func=AF.Identity,
                                         bias=bias_sb["bq"][:, l, c:c + 1])
                else:
                    nc.scalar.copy(out=q_sb[:, c], in_=psq)
                if flags["bk"]:
                    nc.vector.tensor_scalar_add(k_sb[:, c], psk, bias_sb["bk"][:, l, c:c + 1])
                else:
                    nc.vector.tensor_copy(k_sb[:, c], psk)

            ctxT_g = bfp.tile([128, 2, T], BF16, tag="ctxT_g", name=f"ctxT{g}{l}")
            for s in range(4):
                scol = slice(s * K, (s + 1) * K)
                psv = pp.tile([128, D], F32, tag="ps", name=f"psv{g}{l}{s}")
                for ci in range(2):
                    nc.tensor.matmul(psv, r32(xT[:, ci, scol]),
                                     r32(wv_sb[:, l, ci, :]),
                                     start=(ci == 0), stop=(ci == 1))
                psv_h = psv.rearrange("p (h e) -> p h e", h=H)
                v_sb = seqp.tile([128, H, 33], BF16, tag="v_sb", name=f"v{g}{l}{s}")
                if flags["bv"]:
                    nc.vector.tensor_tensor(
                        out=v_sb[:, :, 0:32], in0=psv_h,
                        in1=bias_sb["bv"][:, l].rearrange("p (h e) -> p h e", h=H),
                        op=OP.add)
                else:
                    nc.scalar.copy(out=v_sb[:, :, 0:32], in_=psv_h)
                nc.gpsimd.memset(v_sb[:, :, 32:33], 1.0)

                # logits: PSUM = diag(c_lh) @ hop + K^T Q   (two 4-head tiles)
                pslg_a = pp.tile([128, 4, K], F32, tag="ps", name=f"plga{g}{l}{s}")
                pslg_b = pp.tile([128, 4, K], F32, tag="ps", name=f"plgb{g}{l}{s}")
                for h in range(H):
                    dst = (pslg_a if h < 4 else pslg_b)[:, h % 4, :]
                    hc, hp = h // 4, (h % 4) * 32
                    nc.tensor.matmul(dst, diag_sb[:, l, h, :], hop_bf[:, s, :],
                                     start=True, stop=False, skip_group_check=True)
                    nc.tensor.matmul(dst,
                                     k_sb[hp:hp + 32, hc, scol],
                                     q_sb[hp:hp + 32, hc, scol],
                                     tile_position=(hp, 0),
                                     start=False, stop=True, skip_group_check=True)
                E = seqp.tile([128, H, K], BF16, tag="E", name=f"E{g}{l}{s}")
                nc.scalar.activation(out=E[:, 0:4], in_=pslg_a, func=AF.Exp)
                nc.scalar.activation(out=E[:, 4:8], in_=pslg_b, func=AF.Exp)

                psc = pp.tile([128, H, 33], F32, tag="ps", name=f"psc{g}{l}{s}")
                for h in range(H):
                    nc.tensor.matmul(psc[:, h], E[:, h], v_sb[:, h])
                rs = seqp.tile([128, H], F32, tag="rs", name=f"rs{g}{l}{s}")
                nc.vector.reciprocal(rs, psc[:, :, 32])
                ctx_sb = seqp.tile([128, H, 32], BF16, tag="ctx_sb", name=f"cx{g}{l}{s}")
                nc.vector.scalar_tensor_tensor(
                    out=ctx_sb, in0=psc[:, :, 0:32], scalar=1.0,
                    in1=rs[:, :, None].to_broadcast((128, H, 32)),
                    op0=OP.mult, op1=OP.mult)
                ctx_flat = ctx_sb.rearrange("p h e -> p (h e)")
                for c in range(2):
                    pst = pp.tile([128, 128], BF16, tag="ps", name=f"pst{g}{l}{s}{c}")
                    nc.tensor.transpose(pst, ctx_flat[:, c * 128:(c + 1) * 128],
                                        ident_bf)
                    if c == 0:
                        nc.scalar.copy(out=ctxT_g[:, c, scol], in_=pst)
                    else:
                        nc.vector.tensor_copy(ctxT_g[:, c, scol], pst)
            st["ctxT"] = ctxT_g

        def o_ln1(st, l):
            """u1 = x + Wo ctx; z = g1 * (u1 - mean(u1))  (rstd deferred)."""
            xT, ctxT_g, g = st["xT"], st["ctxT"], st["g"]
            u1 = actp.tile([128, 2, T], F32, tag="u1", name=f"u1_{g}{l}")
            for c in range(2):
                pso = pp.tile([128, T], F32, tag="ps", name=f"pso{g}{l}{c}")
                for ci in range(2):
                    nc.tensor.matmul(pso, wo_sb[:, l, ci, c * 128:(c + 1) * 128],
                                     ctxT_g[:, ci], start=(ci == 0), stop=(ci == 1))
                if flags["bo"]:
                    nc.vector.scalar_tensor_tensor(
                        out=r32(u1[:, c]), in0=pso, scalar=bias_sb["bo"][:, l, c:c + 1],
                        in1=xT[:, c], op0=OP.add, op1=OP.add)
                else:
                    nc.vector.tensor_tensor(out=r32(u1[:, c]), in0=pso, in1=xT[:, c], op=OP.add)
            z = actp.tile([128, 2, T], F32, tag="z", name=f"z_{g}{l}")
            if ln1_fast:
                for c in range(2):
                    psz = pp.tile([128, T], F32, tag="ps", name=f"psz{g}{l}{c}")
                    for ci in range(2):
                        nc.tensor.matmul(psz, r32(jg1_sb[:, l, c, :]),
                                         r32(u1[:, ci]), start=(ci == 0), stop=(ci == 1))
                    nc.vector.scalar_tensor_tensor(
                        out=r32(z[:, c]), in0=u1[:, c], scalar=g1c_sb[:, l, c:c + 1],
                        in1=psz, op0=OP.mult, op1=OP.subtract)
            else:
                layer_norm_full(u1, z, g, l, "g1", g1r_sb,
                                bias_sb["ln1_b"] if flags["ln1_b"] else None)
            st["z"] = z

        def layer_norm_full(u, y, g, l, tag, grow_sb, b_sb):
            """y = (u - m) * (g * rstd) [+ b]; rstd = exp(-0.5 ln(var+eps))."""
            psm = pp.tile([128, T], F32, tag="ps", name=f"psm{tag}{g}{l}")
            for ci in range(2):
                nc.tensor.matmul(psm, r32(jpl), r32(u[:, ci]),
                                 start=(ci == 0), stop=(ci == 1))
            usq = actp.tile([128, 2, T], F32, tag="u1", name=f"usq{g}{l}")
            for ci in range(2):
                nc.scalar.activation(out=r32(usq[:, ci]), in_=u[:, ci], func=AF.Square)
            ps2v = pp.tile([128, T], F32, tag="ps", name=f"ps2v{tag}{g}{l}")
            for ci in range(2):
                nc.tensor.matmul(ps2v[0:1, :], r32(onesd_col), r32(usq[:, ci]),
                                 start=(ci == 0), stop=(ci == 1))
            m2 = smp.tile([1, T], F32, tag="ln_m2")
            nc.scalar.activation(out=m2, in_=psm[0:1, :], func=AF.Square)
            q2 = smp.tile([1, T], F32, tag="ln_q2")
            nc.vector.tensor_tensor(out=q2, in0=ps2v[0:1, :], in1=m2, op=OP.subtract)
            lnv = smp.tile([1, T], F32, tag="ln_q2")
            nc.scalar.activation(out=lnv, in_=q2, func=AF.Ln, bias=eps_sb)
            rstd = smp.tile([1, T], F32, tag="ln_rstd")
            nc.scalar.activation(out=r32(rstd), in_=lnv, func=AF.Exp, scale=-0.5)
            for c in range(2):
                psr = pp.tile([128, T], F32, tag="ps", name=f"psr{tag}{g}{l}{c}")
                nc.tensor.matmul(psr, r32(grow_sb[0:1, l, c, :]), r32(rstd))
                tmp = smp.tile([128, T], F32, tag="ln_tmp")
                nc.vector.tensor_tensor(out=tmp, in0=u[:, c], in1=psm, op=OP.subtract)
                nc.vector.tensor_tensor(out=r32(y[:, c]), in0=tmp, in1=psr, op=OP.mult)
                if b_sb is not None:
                    nc.vector.tensor_scalar_add(r32(y[:, c]), y[:, c], b_sb[:, l, c:c + 1])

        def ffn_ln2(st, l):
            z, g = st["z"], st["g"]
            f1 = bfp.tile([128, 8, T], BF16, tag="f1", name=f"f1_{g}{l}")
            for f in range(8):
                psf = pp.tile([128, T], F32, tag="ps", name=f"psf{g}{l}{f}")
                for ci in range(2):
                    nc.tensor.matmul(psf, r32(w1_sb[:, l, ci, f * 128:(f + 1) * 128]),
                                     r32(z[:, ci]), start=(ci == 0), stop=(ci == 1))
                if flags["b1"]:
                    nc.scalar.activation(out=f1[:, f], in_=psf, func=AF.Relu,
                                         bias=bias_sb["b1"][:, l, f:f + 1])
                elif f < 4:
                    nc.scalar.activation(out=f1[:, f], in_=psf, func=AF.Relu)
                else:
                    nc.vector.tensor_scalar_max(f1[:, f], psf, 0.0)
            u2 = actp.tile([128, 2, T], F32, tag="u2", name=f"u2_{g}{l}")
            for c in range(2):
                ps2 = pp.tile([128, T], F32, tag="ps", name=f"ps2{g}{l}{c}")
                for f in range(8):
                    nc.tensor.matmul(ps2, w2_sb[:, l, f, c * 128:(c + 1) * 128],
                                     f1[:, f], start=(f == 0), stop=(f == 7))
                if flags["b2"]:
                    nc.vector.scalar_tensor_tensor(
                        out=r32(u2[:, c]), in0=ps2, scalar=bias_sb["b2"][:, l, c:c + 1],
                        in1=z[:, c], op0=OP.add, op1=OP.add)
                else:
                    nc.vector.tensor_tensor(out=r32(u2[:, c]), in0=ps2, in1=z[:, c], op=OP.add)
            xT = actp.tile([128, 2, T], F32, tag="xT", name=f"x{g}{l}")
            layer_norm_full(u2, xT, g, l, "g2", g2r_sb,
                            bias_sb["ln2_b"] if flags["ln2_b"] else None)
            st["xT"] = xT

        for g in range(GROUPS):
            st = load_group(g)
            for l in range(L):
                attention(st, l)
                o_ln1(st, l)
                ffn_ln2(st, l)
            nc.gpsimd.tensor_copy(
                r32(xcls[:, :, g * 4:(g + 1) * 4]),
                st["xT"].rearrange("p c (s t) -> p c s t", t=K)[:, :, :, 0])

        # =================== fusion head ===================
        pst = [pp.tile([128, NSEQ], F32, tag="ps", name=f"pstf{c}") for c in range(2)]
        for c in range(2):
            for ci in range(2):
                nc.tensor.matmul(pst[c], r32(fusw_sb[:, ci, c * 128:(c + 1) * 128]),
                                 r32(xcls[:, ci]), start=(ci == 0), stop=(ci == 1))
        th = smp.tile([128, 2, NSEQ], F32, tag="th")
        for c in range(2):
            nc.scalar.activation(out=r32(th[:, c]), in_=pst[c], func=AF.Tanh)
        pssc = pp.tile([128, NSEQ], F32, tag="ps")
        for ci in range(2):
            nc.tensor.matmul(pssc[0:1, :], r32(fusv_sb[:, ci:ci + 1]), r32(th[:, ci]),
                             start=(ci == 0), stop=(ci == 1))
        es = smp.tile([1, NSEQ], F32, tag="es")
        nc.scalar.activation(out=es, in_=pssc[0:1, :], func=AF.Exp)
        esv = es.rearrange("o (b g) -> o b g", g=4)
        s01 = smp.tile([1, B_CORE], F32, tag="s01")
        nc.gpsimd.tensor_tensor(out=s01, in0=esv[:, :, 0], in1=esv[:, :, 1], op=OP.add)
        s23 = smp.tile([1, B_CORE], F32, tag="s23")
        nc.gpsimd.tensor_tensor(out=s23, in0=esv[:, :, 2], in1=esv[:, :, 3], op=OP.add)
        stot = smp.tile([1, B_CORE], F32, tag="stot")
        nc.gpsimd.tensor_tensor(out=stot, in0=s01, in1=s23, op=OP.add)
        rtot = smp.tile([1, B_CORE], F32, tag="rtot")
        nc.vector.reciprocal(rtot, stot)
        w_sm = smp.tile([1, NSEQ], F32, tag="w_sm")
        nc.vector.tensor_tensor(
            out=r32(w_sm.rearrange("o (b g) -> o b g", g=4)), in0=esv,
            in1=rtot[:, :, None].to_broadcast((1, B_CORE, 4)), op=OP.mult)
        pswb = pp.tile([128, NSEQ], F32, tag="ps")
        nc.tensor.matmul(pswb, r32(ones_row), r32(w_sm))
        wx = smp.tile([128, 2, B_CORE, 4], F32, tag="wx")
        nc.vector.tensor_tensor(
            out=wx, in0=xcls.rearrange("p c (b g) -> p c b g", g=4),
            in1=pswb.rearrange("p (b g) -> p b g", g=4)[:, None].to_broadcast(
                (128, 2, B_CORE, 4)),
            op=OP.mult)
        o01 = smp.tile([128, 2, B_CORE], F32, tag="o01")
        nc.gpsimd.tensor_tensor(out=o01, in0=wx[:, :, :, 0], in1=wx[:, :, :, 1], op=OP.add)
        o23 = smp.tile([128, 2, B_CORE], F32, tag="o23")
        nc.gpsimd.tensor_tensor(out=o23, in0=wx[:, :, :, 2], in1=wx[:, :, :, 3], op=OP.add)
        outT = smp.tile([128, 2, B_CORE], F32, tag="outT")
        nc.gpsimd.tensor_tensor(out=outT, in0=o01, in1=o23, op=OP.add)
        nc.sync.dma_start(out=out_d, in_=outT)

    with _act_steering():
        nc.compile()
    return nc


# ======================= host side =======================

def host_prep(inputs):
    """Full inputs -> (flags, per-core in_maps list)."""
    f32 = np.float32
    node_id = np.asarray(inputs["node_id"])
    nbr_tab = np.asarray(inputs["neighbor_table"])
    deg_tab = np.asarray(inputs["degree_table"])
    feat_tab = np.asarray(inputs["node_feat_table"], dtype=f32)
    centr_tab = np.asarray(inputs["centr_table"], dtype=f32)
    sp_tab = np.asarray(inputs["spatial_table"], dtype=f32)

    spW1 = np.asarray(inputs["spW1"], dtype=f32)
    spb1 = np.asarray(inputs["spb1"], dtype=f32)
    spW2 = np.asarray(inputs["spW2"], dtype=f32)
    assert np.all(spb1 == 0.0), "kernel assumes spb1 == 0 (as in setup_inputs)"
    c_coef = np.einsum("ld,ldh->lh", np.maximum(spW1[:, 0, :], 0.0), spW2)  # [L,H]

    Wq = np.asarray(inputs["Wq"], dtype=f32) / np.sqrt(np.float32(DH))
    bq = np.asarray(inputs["bq"], dtype=f32) / np.sqrt(np.float32(DH))
    Wk = np.asarray(inputs["Wk"], dtype=f32)
    Wv = np.asarray(inputs["Wv"], dtype=f32)
    Wo = np.asarray(inputs["Wo"], dtype=f32).astype(ml_dtypes.bfloat16)
    W1 = np.asarray(inputs["ffn_W1"], dtype=f32)
    W2 = np.asarray(inputs["ffn_W2"], dtype=f32).astype(ml_dtypes.bfloat16)
    g1 = np.asarray(inputs["ln1_g"], dtype=f32)
    g2 = np.asarray(inputs["ln2_g"], dtype=f32)
    linW = np.asarray(inputs["lin_W"], dtype=f32)

    def pmaj(v, cols):   # [X] -> [128, X/128] partition-major
        return np.ascontiguousarray(v.reshape(cols, 128).T)

    def pmaj_l(v, cols):  # [L, X] -> [128, L, X/128]
        return np.ascontiguousarray(v.reshape(L, cols, 128).transpose(2, 0, 1))

    flags = {
        "lin_b": bool(np.any(np.asarray(inputs["lin_b"]) != 0)),
        "bq": bool(np.any(bq != 0)),
        "bk": bool(np.any(np.asarray(inputs["bk"]) != 0)),
        "bv": bool(np.any(np.asarray(inputs["bv"]) != 0)),
        "bo": bool(np.any(np.asarray(inputs["bo"]) != 0)),
        "b1": bool(np.any(np.asarray(inputs["ffn_b1"]) != 0)),
        "b2": bool(np.any(np.asarray(inputs["ffn_b2"]) != 0)),
        "ln1_b": bool(np.any(np.asarray(inputs["ln1_b"]) != 0)),
        "ln2_b": bool(np.any(np.asarray(inputs["ln2_b"]) != 0)),
    }

    shared = {
        "lin_W": np.ascontiguousarray(linW),
        "lin_W16": np.ascontiguousarray(linW * np.sqrt(np.float32(D))),
        "Wq": np.ascontiguousarray(Wq), "Wk": np.ascontiguousarray(Wk),
        "Wv": np.ascontiguousarray(Wv), "Wo": np.ascontiguousarray(Wo),
        "W1": np.ascontiguousarray(W1), "W2": np.ascontiguousarray(W2),
        "fus_W": np.ascontiguousarray(np.asarray(inputs["fus_W"], dtype=f32)),
        "fus_v": pmaj(np.asarray(inputs["fus_v"], dtype=f32), 2),
        "csb": np.ascontiguousarray(
            np.broadcast_to(c_coef[None, :, :], (128, L, H)).astype(f32)),
        "Jg1": np.ascontiguousarray(np.broadcast_to(
            (g1 / np.float32(D)).reshape(L, 2, 128)[None], (128, L, 2, 128)).astype(f32)),
        "g2row": np.ascontiguousarray(g2.reshape(1, L, 2, 128).astype(f32)),
        "g1row": np.ascontiguousarray(g1.reshape(1, L, 2, 128).astype(f32)),
        "g1col": pmaj_l(g1, 2),
    }
    if flags["lin_b"]:
        shared["lin_b"] = pmaj(np.asarray(inputs["lin_b"], dtype=f32), 2)
    if flags["bq"]:
        shared["bq"] = pmaj_l(bq, 2)
    if flags["bk"]:
        shared["bk"] = pmaj_l(np.asarray(inputs["bk"], dtype=f32), 2)
    if flags["bv"]:
        shared["bv"] = np.ascontiguousarray(np.asarray(inputs["bv"], dtype=f32))
    if flags["bo"]:
        shared["bo"] = pmaj_l(np.asarray(inputs["bo"], dtype=f32), 2)
    if flags["b1"]:
        shared["b1"] = pmaj_l(np.asarray(inputs["ffn_b1"], dtype=f32), 8)
    if flags["b2"]:
        shared["b2"] = pmaj_l(np.asarray(inputs["ffn_b2"], dtype=f32), 2)
    if flags["ln1_b"]:
        shared["ln1_b"] = pmaj_l(np.asarray(inputs["ln1_b"], dtype=f32), 2)
    if flags["ln2_b"]:
        shared["ln2_b"] = pmaj_l(np.asarray(inputs["ln2_b"], dtype=f32), 2)

    in_maps = []
    for core in range(8):
        ids = node_id[core * B_CORE:(core + 1) * B_CORE]
        idx = nbr_tab[ids].reshape(-1)                       # [4096]
        featT = np.ascontiguousarray(np.nan_to_num(feat_tab[idx]).T)   # [256,4096]
        deg = deg_tab[idx, 0]
        centrT = np.ascontiguousarray(centr_tab[deg].T)
        hopT = np.ascontiguousarray(
            sp_tab[ids][:, :, 0].transpose(0, 1, 3, 2).reshape(NSEQ, K, K)
        ).astype(ml_dtypes.bfloat16)
        m = dict(shared)
        m["featT"] = featT
        m["centrT"] = centrT
        m["hopT"] = hopT
        in_maps.append(m)
    return flags, in_maps


def assemble(results):
    """per-core outT [128, 2, 8] -> full [64, 256] f32."""
    outs = []
    for core in range(8):
        oT = results[core]["outT"]                # [128, 2, 8]
        outs.append(oT.transpose(2, 1, 0).reshape(B_CORE, D))
    return np.ascontiguousarray(np.concatenate(outs, 0).astype(np.float32))


# ======================= entry point =======================

import os as _os
_os.environ.setdefault("NEURON_RT_RESET_CORES", "1")

_BUILD_CACHE = {}


def kernel(**inputs):
    """Full (unsharded) inputs -> full [64, 256] float32 output."""
    from concourse import bass_utils
    flags, in_maps = host_prep(inputs)
    key = tuple(sorted((k, v) for k, v in flags.items()))
    if key not in _BUILD_CACHE:
        _BUILD_CACHE[key] = build_nc(flags)
    nc = _BUILD_CACHE[key]
    res = bass_utils.run_bass_kernel_spmd(nc, in_maps, core_ids=list(range(8)))
    return assemble(res.results)
